# revision 16
# baseline (speedup 1.0000x reference)
"""Variable-length average pooling (prefix mean over seq axis) on 8 trn2 cores.

Strategy (pure data parallelism over batch):
  - eff_len[b] = lengths[b] if >0 else L.  pooled[b] = sum_{l<eff} x[b,l,:] / eff.
  - Sort batches by eff_len desc, snake-assign 16 per core so per-core work and
    per-slot length profiles are balanced across cores (~0.8% imbalance).
  - One SPMD Bass program shared by all 8 cores: slot j processes
    ceil(max_core_len_j/128) L-chunks of [rows<=128, 2048]; rows beyond a
    core's own length are zeroed by the per-core mask weights, so only the
    slot-max structure is baked into the program (+5% extra DMA vs ideal).
  - fp32 moving operands run the PE at 1/4 rate, which would make PE the
    bottleneck (~293us busy vs ~240us DMA), so the reduction is split:
      * "uniform" chunks (all 128 rows valid on every core, i.e.
        128*(k+1) <= min_core_len) are summed on the VectorE into an SBUF
        accumulator (tensor_tensor add, full fp32), then reduced across
        partitions by one PE matmul against a 1/len column.
      * ragged chunks go straight to the PE as
        psum[1,512] += maskcol[rows,1].T @ tile[rows,512],
        maskcol[p] = (128k+p < eff)/eff (scale folded in).
  - PSUM [1,2048] -> SBUF via ScalarE copy -> DMA out.
"""

import os

import numpy as np

import concourse.bacc as bacc
import concourse.mybir as mybir
from concourse.tile import TileContext
from concourse.bass_utils import run_bass_kernel_spmd

B, L, D = 128, 1024, 2048
NCORES = 8
SLOTS = B // NCORES  # 16
PCHUNK = 128         # L-rows per chunk (partition dim of the tile)
MAXK = L // PCHUNK   # 8
NTILE = 512          # matmul moving free dim (one PSUM bank of fp32)
MCOLS = SLOTS * MAXK + SLOTS  # mask columns + per-slot 1/len columns

TILE_BUFS = int(os.environ.get("TILE_BUFS", "8"))

LAST_RESULTS = None  # BassKernelResults of the most recent device run


def _plan(eff):
    """Snake-assign sorted batches to cores.

    Returns (cores[c][s] -> batch idx, slot_rows[s] -> per-chunk row counts,
    slot_uniform[s] -> #leading chunks full on every core)."""
    order = np.argsort(-eff, kind="stable")
    cores = [[] for _ in range(NCORES)]
    for i, idx in enumerate(order):
        blk, pos = divmod(i, NCORES)
        c = pos if blk % 2 == 0 else NCORES - 1 - pos
        cores[c].append(int(idx))
    slot_rows, slot_uniform = [], []
    for s in range(SLOTS):
        lens = [int(eff[cores[c][s]]) for c in range(NCORES)]
        m, mn = max(lens), min(lens)
        nk = -(-m // PCHUNK)
        slot_rows.append(tuple(min(PCHUNK, m - PCHUNK * k) for k in range(nk)))
        slot_uniform.append(mn // PCHUNK)
    return cores, tuple(slot_rows), tuple(slot_uniform)


_PROGRAM_CACHE = {}


def _build_program(slot_rows, slot_uniform):
    # Bacc (not raw Bass): its compile pass splits multi-sem waits and moves
    # matmul waits onto ldweights — walrus allows only 1 wait per instruction.
    nc = bacc.Bacc(None, target_bir_lowering=False)
    f32 = mybir.dt.float32
    feat = nc.dram_tensor("features", [SLOTS, L, D], f32, kind="ExternalInput")
    maskt = nc.dram_tensor("maskt", [PCHUNK, MCOLS], f32, kind="ExternalInput")
    out = nc.dram_tensor("out", [SLOTS, D], f32, kind="ExternalOutput")

    with TileContext(nc) as tc:
        with (
            tc.tile_pool(name="mask", bufs=1) as mpool,
            tc.tile_pool(name="tiles", bufs=TILE_BUFS) as tpool,
            tc.tile_pool(name="accs", bufs=3) as apool,
            tc.tile_pool(name="psum", bufs=4, space="PSUM") as ppool,
            tc.tile_pool(name="outs", bufs=3) as opool,
        ):
            mask_tile = mpool.tile([PCHUNK, MCOLS], f32)
            nc.sync.dma_start(out=mask_tile[:], in_=maskt[:])
            # Alternate the two HWDGE rings (SP + ACT) for the big loads:
            # measured 318 -> ~390 GB/s vs a single ring.
            dma_engines = [nc.sync, nc.scalar]
            n_dma = 0
            for s in range(SLOTS):
                rows_list = slot_rows[s]
                nk = len(rows_list)
                nu = slot_uniform[s]
                psum_a = ppool.tile([1, D // 2], f32, name="psum_a", tag="ps")
                psum_b = ppool.tile([1, D // 2], f32, name="psum_b", tag="ps")
                psum_half = [psum_a, psum_a, psum_b, psum_b]
                acc = (
                    apool.tile([PCHUNK, D], f32, name="acc", tag="acc")
                    if nu > 0
                    else None
                )

                # Load L-chunks in 2 MB pairs [128, 2D] (chunk halves side by
                # side) over the full chunks; leftover full chunk as a 1 MB
                # single; the slot's partial last chunk as a row-trimmed
                # single so no padded rows are fetched.
                halves = {}  # chunk k -> (tile, col offset, rows)
                n_full = nk
                k = 0
                while k < n_full:
                    if k + 1 < n_full:
                        pair = tpool.tile([PCHUNK, 2 * D], f32, name="pair", tag="t")
                        src = feat[s, k * PCHUNK : (k + 2) * PCHUNK, :].rearrange(
                            "(c p) d -> p c d", p=PCHUNK
                        )
                        dst = pair[:].rearrange("p (c d) -> p c d", c=2)
                        dma_engines[n_dma % 2].dma_start(out=dst, in_=src)
                        halves[k] = (pair, 0, PCHUNK)
                        halves[k + 1] = (pair, D, PCHUNK)
                        k += 2
                    else:
                        single = tpool.tile([PCHUNK, D], f32, name="single", tag="t")
                        dma_engines[n_dma % 2].dma_start(
                            out=single[:], in_=feat[s, k * PCHUNK : (k + 1) * PCHUNK, :]
                        )
                        halves[k] = (single, 0, PCHUNK)
                        k += 1
                    n_dma += 1
                if n_full < nk:
                    rows = rows_list[-1]
                    ptile = tpool.tile([PCHUNK, D], f32, name="ptile", tag="t")
                    dma_engines[n_dma % 2].dma_start(
                        out=ptile[:rows],
                        in_=feat[s, n_full * PCHUNK : n_full * PCHUNK + rows, :],
                    )
                    halves[n_full] = (ptile, 0, rows)
                    n_dma += 1

                # VectorE path: full-on-every-core chunks, plain fp32 adds.
                # Pairs whose two halves are both uniform get an in-place
                # pair-sum (independent of the chain, frees the DMA pipeline);
                # the accumulator chain then combines the pair-sums.
                nodes = []
                k = 0
                while k < nu:
                    tile, off, _ = halves[k]
                    if k + 1 < nu and halves[k + 1][0] is tile:
                        nc.vector.tensor_add(
                            out=tile[:, 0:D], in0=tile[:, 0:D], in1=tile[:, D : 2 * D]
                        )
                        nodes.append((tile, 0))
                        k += 2
                    else:
                        nodes.append((tile, off))
                        k += 1
                if len(nodes) == 1:
                    t0, o0 = nodes[0]
                    nc.vector.tensor_copy(out=acc[:], in_=t0[:, o0 : o0 + D])
                elif nodes:
                    t0, o0 = nodes[0]
                    t1, o1 = nodes[1]
                    nc.vector.tensor_add(
                        out=acc[:], in0=t0[:, o0 : o0 + D], in1=t1[:, o1 : o1 + D]
                    )
                    for tn, on in nodes[2:]:
                        nc.vector.tensor_add(
                            out=acc[:], in0=acc[:], in1=tn[:, on : on + D]
                        )

                # PE path: ragged chunks, per-core mask/len weights.
                n_mm = (nk - nu) + (1 if nu > 0 else 0)  # accumulation group size
                mm_i = 0
                for k in range(nu, nk):
                    tile, off, rows = halves[k]
                    col = s * MAXK + k
                    for j in range(D // NTILE):
                        nc.tensor.matmul(
                            psum_half[j][0:1, (j % 2) * NTILE : (j % 2 + 1) * NTILE],
                            mask_tile[0:rows, col : col + 1],
                            tile[0:rows, off + j * NTILE : off + (j + 1) * NTILE],
                            start=(mm_i == 0),
                            stop=(mm_i == n_mm - 1),
                        )
                    mm_i += 1

                # Cross-partition reduce of the DVE accumulator: 1/len column.
                if nu > 0:
                    col = SLOTS * MAXK + s
                    for j in range(D // NTILE):
                        nc.tensor.matmul(
                            psum_half[j][0:1, (j % 2) * NTILE : (j % 2 + 1) * NTILE],
                            mask_tile[:, col : col + 1],
                            acc[:, j * NTILE : (j + 1) * NTILE],
                            start=(mm_i == 0),
                            stop=True,
                        )

                # DVE (not ACT) for the PSUM->SBUF copy: the ACT sequencer
                # issues half the loads, and a copy queued behind a stalled
                # DMA issue would delay the PSUM release and stall the PE.
                out_t = opool.tile([1, D], f32)
                nc.vector.tensor_copy(out=out_t[:, 0 : D // 2], in_=psum_a[:])
                nc.vector.tensor_copy(out=out_t[:, D // 2 : D], in_=psum_b[:])
                nc.sync.dma_start(out=out[s : s + 1, :], in_=out_t[:])
    nc.finalize()
    return nc


def kernel(features, lengths):
    global LAST_RESULTS
    features = np.ascontiguousarray(features, dtype=np.float32)
    lengths = np.ascontiguousarray(lengths, dtype=np.int32)
    eff = np.where(lengths > 0, lengths, L).astype(np.int64)

    cores, slot_rows, slot_uniform = _plan(eff)
    key = (slot_rows, slot_uniform, TILE_BUFS)
    if key not in _PROGRAM_CACHE:
        _PROGRAM_CACHE[key] = _build_program(slot_rows, slot_uniform)
    nc = _PROGRAM_CACHE[key]

    in_maps = []
    for c in range(NCORES):
        perm = cores[c]
        maskt = np.zeros((PCHUNK, MCOLS), dtype=np.float32)
        for s, b in enumerate(perm):
            e = int(eff[b])
            inv = np.float32(1.0 / e)
            for k in range(slot_uniform[s], len(slot_rows[s])):
                lo = k * PCHUNK
                n_valid = min(max(e - lo, 0), PCHUNK)
                if n_valid > 0:
                    maskt[:n_valid, s * MAXK + k] = inv
            maskt[:, SLOTS * MAXK + s] = inv
        in_maps.append({"features": features[perm], "maskt": maskt})

    trace = os.environ.get("KERNEL_TRACE", "0") == "1"
    LAST_RESULTS = run_bass_kernel_spmd(
        nc,
        in_maps,
        core_ids=list(range(NCORES)),
        trace=trace,
        trace_cores=[0] if trace else None,
    )

    out = np.empty((B, D), dtype=np.float32)
    for c in range(NCORES):
        out[np.asarray(cores[c])] = LAST_RESULTS.results[c]["out"]
    return out
